# revision 31
# baseline (speedup 1.0000x reference)
"""Sparse (block-diagonal) multi-head attention kernel for Trainium2.

Problem: nn_MultiHeadAttention_75161927680550
  x_q, x_kv: [4096, 256] f32; batch_q, batch_kv: [4096] sorted int64 in [0,16)
  torch-style MHA with 8 heads, d_k=32; attention is masked to
  batch_q[i] == batch_kv[j], which (ids sorted) is block-diagonal over <=16
  segments.  Returns (out [4096,256], attn [1,8,4096,4096]).

Strategy (head-parallel SPMD over 8 cores, one head per core):
  - Host computes segment boundaries from the batch-id vectors and builds a
    Bass/Tile program specialized to them (compile happens once per boundary
    signature and is cached).
  - Device (per core h) computes, entirely in a transposed ("k-major") layout
    that needs no on-chip transposes:
      QT = Wq_h @ x_q^T + bq_h            [32, 4096]
      KT = Wk_h @ x_kv^T + bk_h           [32, 4096]
      V' = [x_kv @ Wv_h^T | 1]            [4096, 33] (ones column -> softmax
                                           denominators fall out of the AV
                                           matmul for free)
      per segment s, per 128-row kv chunk:
        scoresT = K_s_chunk @ Q_s^T       (PE, k-major)
        u = exp(scoresT / sqrt(32))       (ACT; scores are O(0.5) so the
                                           max-subtraction is unnecessary)
        DMA u -> attnT output (unnormalized, k-major)
        o'T += V'_chunk^T @ u             (PE accumulate -> [33, n_q])
      poutT = G_h @ oT[0:32]              [256, 4096] (G_h = Wo[:, 32h:32h+32])
      den   = oT[32]                      [4096]
  - Host: rinv = 1/den; scatters normalized attention blocks into a zeroed
    [8,4096,4096] (off-block entries are exactly 0 in the reference); output
    rows scale by rinv before the head-sum (diagonal scaling commutes with
    the output projection); biases bv/bo fold into a host-side constant row
    because softmax rows sum to one.
"""

import numpy as np

import concourse.bass as bass
import concourse.mybir as mybir
import concourse.tile as tile
from concourse import bacc
from concourse.bass import ts
from concourse.bass_utils import run_bass_kernel_spmd

N = 4096
D_MODEL = 256
N_HEADS = 8
D_K = 32
N_SEG = 16
N_CORES = 8
F32 = mybir.dt.float32
F32R = mybir.dt.float32r
BF16 = mybir.dt.bfloat16
# matmul datapath precision: "f32r" (~tf32, rel err ~3e-4) or "bf16"
# (faster: FWL weight loads + half the x/attn DMA bytes, rel err ~5e-3)
PRECISION = "f32r"
MDT = BF16 if PRECISION == "bf16" else F32R


# stash of the most recent BassKernelResults (test.py reads exec_time_ns)
LAST_RESULTS = None

_PROGRAM_CACHE = {}


def _segments(batch_q, batch_kv):
    """Per-segment [start, end) ranges for q and kv row spaces."""
    bq = np.asarray(batch_q).astype(np.int64)
    bk = np.asarray(batch_kv).astype(np.int64)
    assert (np.diff(bq) >= 0).all() and (np.diff(bk) >= 0).all(), (
        "batch ids must be sorted"
    )
    qb, kb = [], []
    for s in range(N_SEG):
        q0, q1 = np.searchsorted(bq, [s, s + 1])
        k0, k1 = np.searchsorted(bk, [s, s + 1])
        if q1 > q0:
            assert k1 > k0, f"segment {s} has queries but no keys (reference NaNs)"
        qb.append((int(q0), int(q1)))
        kb.append((int(k0), int(k1)))
    return tuple(qb), tuple(kb)


def _build_program(qb, kb, use_bias):
    """Build + schedule the per-core Bass program for the given boundaries."""
    kv_chunks = []  # (seg, abs_start, nck)
    seg_chunk_ids = {s: [] for s in range(N_SEG)}
    for s in range(N_SEG):
        k0, k1 = kb[s]
        off = k0
        while off < k1:
            nck = min(128, k1 - off)
            seg_chunk_ids[s].append(len(kv_chunks))
            kv_chunks.append((s, off, nck))
            off += nck
    n_kv_chunks = len(kv_chunks)
    # flat job list: (seg, qoff, qn); q chunks <=512 for psum/moving limits
    jobs = []
    attn_cols = 0
    for s in range(N_SEG):
        q0, q1 = qb[s]
        off = 0
        while off < q1 - q0:
            qn = min(512, q1 - q0 - off)
            jobs.append((s, off, qn))
            off += qn
        attn_cols = max(attn_cols, q1 - q0)
    QCOLS = attn_cols

    nc = bacc.Bacc("TRN2", target_bir_lowering=False, debug=False,
                   num_devices=N_CORES)

    xqT = nc.dram_tensor("xqT", (D_MODEL, N), MDT, kind="ExternalInput").ap()
    xkvT = nc.dram_tensor("xkvT", (D_MODEL, N), MDT, kind="ExternalInput").ap()
    # packed weights: [WqT | WkT | WvT] and [gT | bq | bk]
    wT3 = nc.dram_tensor("wT3", (D_MODEL, 3 * D_K), MDT, kind="ExternalInput").ap()
    gT = nc.dram_tensor("gT", (D_K, D_MODEL), MDT, kind="ExternalInput").ap()
    bqk = nc.dram_tensor("bqk", (D_K, 2), F32, kind="ExternalInput").ap()

    attnT = nc.dram_tensor("attnT", (N, QCOLS), MDT, kind="ExternalOutput").ap()
    poutT = nc.dram_tensor("poutT", (D_MODEL, N), F32, kind="ExternalOutput").ap()
    den = nc.dram_tensor("den", (1, N), F32, kind="ExternalOutput").ap()

    SCALE = float(1.0 / np.sqrt(np.float32(D_K)))
    Exp = mybir.ActivationFunctionType.Exp
    XC = 512  # x DMA chunk width

    with tile.TileContext(nc) as tc:
        with (
            tc.tile_pool(name="big", bufs=1) as big,
            tc.tile_pool(name="wp", bufs=1) as wp,
            tc.tile_pool(name="work", bufs=12) as work,
            tc.tile_pool(name="fop", bufs=4) as fop,
            tc.tile_pool(name="psmm", bufs=6, space="PSUM") as psmm,
            tc.tile_pool(name="psacc", bufs=2, space="PSUM") as psacc,
        ):
            # ---- proj weights first; g/bqk deferred behind early x chunks
            # (g is first needed by the fin phase ~40us in; bqk only if bias)
            w3_s = wp.tile([128, 2, 3 * D_K], MDT, tag="w3")
            nc.sync.dma_start(w3_s[:], wT3.rearrange("(c p) m -> p c m", p=128))
            g_s = wp.tile([D_K, D_MODEL], MDT, tag="g")
            if use_bias:
                bqk_s = wp.tile([D_K, 2], F32, tag="bqk")
                nc.scalar.dma_start(bqk_s[:], bqk[:, :])
                bq_s = bqk_s[:, 0:1]
                bk_s = bqk_s[:, 1:2]
            else:
                bq_s = bk_s = None

            # ---- x loads, chunked + interleaved so projections start early
            xq_s = big.tile([128, 2, N], MDT, tag="xq")
            xkv_s = big.tile([128, 2, N], MDT, tag="xkv")
            xqr = xqT.rearrange("(c p) n -> p c n", p=128)
            xkvr = xkvT.rearrange("(c p) n -> p c n", p=128)
            xsplits = [0, 512, 1024, 1536, 2048, 3072, 4096]
            for ci in range(len(xsplits) - 1):
                sl = slice(xsplits[ci], xsplits[ci + 1])
                nc.sync.dma_start(xq_s[:, :, sl], xqr[:, :, sl])
                nc.scalar.dma_start(xkv_s[:, :, sl], xkvr[:, :, sl])
                if ci == 1:
                    nc.scalar.dma_start(g_s[:], gT[:, :])

            # ---- Q^T, K^T projections: [32, 4096(+2 zero-pad cols)] ----
            # fp32r matmuls need an even moving-free count; odd segment
            # sizes get padded by one column, so zero the tail columns.
            qT = big.tile([D_K, N + 2], MDT, tag="qT")
            kT = big.tile([D_K, N + 2], MDT, tag="kT")
            zt = wp.tile([D_K, 2], F32, tag="zt")
            nc.vector.memset(zt[:], 0.0)
            nc.vector.tensor_copy(qT[:, N : N + 2], zt[:])
            nc.vector.tensor_copy(kT[:, N : N + 2], zt[:])

            vch = big.tile([128, n_kv_chunks, D_K + 1], MDT, tag="v")
            ones = wp.tile([128, 1], F32, tag="ones")
            nc.vector.memset(ones[:], 1.0)
            nc.vector.tensor_copy(
                vch[:, :, D_K : D_K + 1],
                ones[:, 0:1, None].to_broadcast((128, n_kv_chunks, 1)),
            )
            oT = big.tile([D_K + 1, N], MDT, tag="oT")
            den_sb = big.tile([1, N], F32, tag="den")
            ndma = 0

            def emit_proj(n):
                for dst, wsl, b_s, x_s in (
                    (qT, slice(0, D_K), bq_s, xq_s),
                    (kT, slice(D_K, 2 * D_K), bk_s, xkv_s),
                ):
                    ps = psmm.tile([128, 512], F32, tag="mm")
                    for c in range(2):
                        nc.tensor.matmul(
                            ps[:D_K, :],
                            w3_s[:, c, wsl],
                            x_s[:, c, ts(n, 512)],
                            start=(c == 0),
                            stop=(c == 1),
                        )
                    if use_bias:
                        nc.vector.tensor_scalar_add(
                            dst[:, ts(n, 512)], ps[:D_K, :], b_s
                        )
                    else:
                        nc.vector.tensor_copy(dst[:, ts(n, 512)], ps[:D_K, :])

            def emit_vchunk(j):
                s, a0, nck = kv_chunks[j]
                ps = psmm.tile([128, 512], F32, tag="mm")
                for c in range(2):
                    nc.tensor.matmul(
                        ps[:nck, :D_K],
                        xkv_s[:, c, a0 : a0 + nck],
                        w3_s[:, c, 2 * D_K : 3 * D_K],
                        start=(c == 0),
                        stop=(c == 1),
                    )
                nc.vector.tensor_copy(vch[:nck, j, :D_K], ps[:nck, :D_K])

            def emit_scores(job):
                s, qoff, qn = job
                q0 = qb[s][0]
                qn_p = qn + (qn & 1)
                u_tiles = []
                nonlocal ndma
                for j in seg_chunk_ids[s]:
                    _, a0, nck = kv_chunks[j]
                    pscore = psmm.tile([128, 512], F32, tag="mm")
                    nc.tensor.matmul(
                        pscore[:nck, :qn_p],
                        kT[:, a0 : a0 + nck],
                        qT[:, q0 + qoff : q0 + qoff + qn_p],
                        start=True,
                        stop=True,
                    )
                    u = work.tile([128, 512], MDT, tag="u")
                    nc.scalar.activation(
                        u[:nck, :qn_p], pscore[:nck, :qn_p], Exp, scale=SCALE
                    )
                    # keep ACT DMA-free early (exp gates AV on PE); at the
                    # tail sync carries the final pout blocks, so the last
                    # jobs' stores go to scalar instead
                    dma_eng = nc.scalar if a0 >= 3 * N // 4 else nc.sync
                    ndma += 1
                    dma_eng.dma_start(
                        attnT[a0 : a0 + nck, qoff : qoff + qn], u[:nck, :qn]
                    )
                    u_tiles.append((u, nck, j))
                return u_tiles

            def emit_av(job, u_tiles):
                s, qoff, qn = job
                q0 = qb[s][0]
                qn_p = qn + (qn & 1)
                po = psacc.tile([D_K + 1, 512], F32, tag="acc")
                last = len(u_tiles) - 1
                for ci, (u, nck, j) in enumerate(u_tiles):
                    nc.tensor.matmul(
                        po[:, :qn_p],
                        vch[:nck, j, :],
                        u[:nck, :qn_p],
                        start=(ci == 0),
                        stop=(ci == last),
                    )
                nc.vector.tensor_copy(
                    oT[:, q0 + qoff : q0 + qoff + qn], po[:, :qn]
                )
                nc.vector.tensor_copy(
                    den_sb[0:1, q0 + qoff : q0 + qoff + qn],
                    po[D_K : D_K + 1, :qn],
                )

            def emit_fin(n):
                for mo in range(2):
                    pf = psmm.tile([128, 512], F32, tag="mm")
                    nc.tensor.matmul(
                        pf[:],
                        g_s[:, mo * 128 : (mo + 1) * 128],
                        oT[:D_K, ts(n, 512)],
                        start=True,
                        stop=True,
                    )
                    ot = fop.tile([128, 512], F32, tag="fout")
                    nc.vector.tensor_copy(ot[:], pf[:])
                    peng = nc.sync if (n + mo) % 2 == 0 else nc.scalar
                    peng.dma_start(
                        poutT[mo * 128 : (mo + 1) * 128, ts(n, 512)], ot[:]
                    )

            # a fin block [512n, 512(n+1)) can run after the last AV whose
            # q-range touches it; AV for job i is emitted one step behind
            # (software pipeline), so fire fin when job i+1 has been emitted.
            last_job_touching = {}
            for i, (s, qoff, qn) in enumerate(jobs):
                q0 = qb[s][0]
                for n in range(
                    (q0 + qoff) // 512, (q0 + qoff + qn - 1) // 512 + 1
                ):
                    last_job_touching[n] = i
            DEPTH = 1  # scores run this many jobs ahead of their AV

            # interleave proj / V' / attention per 512-col block so the PE
            # always has ready work while x chunks stream in (keeps the
            # HAM clock-gate warm)
            vch_done = 0
            job_idx = 0
            pending = []
            avs_done = 0
            fin_done = set()

            def maybe_fin():
                # all blocks whose last-touching job's AV has been emitted
                for n in range(N // 512):
                    if n in fin_done:
                        continue
                    if last_job_touching.get(n, -1) < avs_done:
                        fin_done.add(n)
                        emit_fin(n)

            def push_job(job):
                nonlocal avs_done
                pending.append((job, emit_scores(job)))
                if len(pending) > DEPTH:
                    emit_av(*pending.pop(0))
                    avs_done += 1
                    maybe_fin()

            for n in range(N // 512):
                emit_proj(n)
                hi = 512 * (n + 1)
                while vch_done < n_kv_chunks:
                    s, a0, nck = kv_chunks[vch_done]
                    if a0 + nck > hi:
                        break
                    emit_vchunk(vch_done)
                    vch_done += 1
                while job_idx < len(jobs):
                    s, qoff, qn = jobs[job_idx]
                    if qb[s][1] > hi or kb[s][1] > hi:
                        break
                    push_job(jobs[job_idx])
                    job_idx += 1
            while job_idx < len(jobs):
                push_job(jobs[job_idx])
                job_idx += 1
            while pending:
                emit_av(*pending.pop(0))
                avs_done += 1
                maybe_fin()

            nc.sync.dma_start(den[0:1, :], den_sb[0:1, :])

    nc.compile()
    return nc, QCOLS


def kernel(x_q, x_kv, batch_q, batch_kv, Wq, bq, Wk, bk, Wv, bv, Wo, bo):
    global LAST_RESULTS
    x_q = np.ascontiguousarray(np.asarray(x_q, dtype=np.float32))
    x_kv = np.ascontiguousarray(np.asarray(x_kv, dtype=np.float32))
    Wq = np.asarray(Wq, dtype=np.float32)
    Wk = np.asarray(Wk, dtype=np.float32)
    Wv = np.asarray(Wv, dtype=np.float32)
    Wo = np.asarray(Wo, dtype=np.float32)
    bq = np.asarray(bq, dtype=np.float32)
    bk = np.asarray(bk, dtype=np.float32)
    bv = np.asarray(bv, dtype=np.float32)
    bo = np.asarray(bo, dtype=np.float32)

    qb, kb = _segments(batch_q, batch_kv)
    use_bias = bool(np.any(bq) or np.any(bk))
    key = (qb, kb, use_bias)
    if key not in _PROGRAM_CACHE:
        _PROGRAM_CACHE[key] = _build_program(qb, kb, use_bias)
    nc, qcols = _PROGRAM_CACHE[key]

    import ml_dtypes

    mnp = ml_dtypes.bfloat16 if PRECISION == "bf16" else np.float32
    xqT = np.ascontiguousarray(x_q.T).astype(mnp)
    xkvT = np.ascontiguousarray(x_kv.T).astype(mnp)
    in_maps = []
    for h in range(N_CORES):
        sl = slice(h * D_K, (h + 1) * D_K)
        wT3 = np.ascontiguousarray(
            np.concatenate([Wq[sl, :].T, Wk[sl, :].T, Wv[sl, :].T], axis=1)
        ).astype(mnp)
        gT = np.ascontiguousarray(Wo[:, sl].T).astype(mnp)
        bqk = np.ascontiguousarray(
            np.stack([bq[sl], bk[sl]], axis=1).astype(np.float32)
        )
        in_maps.append(
            {"xqT": xqT, "xkvT": xkvT, "wT3": wT3, "gT": gT, "bqk": bqk}
        )

    res = run_bass_kernel_spmd(nc, in_maps, core_ids=list(range(N_CORES)))
    LAST_RESULTS = res

    # ---- host-side assembly ----
    out = np.zeros((N, D_MODEL), dtype=np.float32)
    attn = np.zeros((N_HEADS, N, N), dtype=np.float32)
    for h in range(N_CORES):
        r = res.results[h]
        den = r["den"].reshape(N)
        poutT = r["poutT"]  # [256, 4096]
        attnT = np.asarray(r["attnT"], dtype=np.float32)  # [4096, qcols]
        with np.errstate(divide="ignore"):
            rinv = np.where(den != 0.0, 1.0 / den, 0.0).astype(np.float32)
        out += (poutT * rinv[None, :]).T
        for s in range(N_SEG):
            q0, q1 = qb[s]
            k0, k1 = kb[s]
            if q1 <= q0 or k1 <= k0:
                continue
            blk = attnT[k0:k1, : q1 - q0].T * rinv[q0:q1, None]
            attn[h, q0:q1, k0:k1] = blk
    bo_eff = bo + bv @ Wo.T
    out += bo_eff[None, :]
    return out, attn[None]


# revision 32
# speedup vs baseline: 1.0032x; 1.0032x over previous
"""Sparse (block-diagonal) multi-head attention kernel for Trainium2.

Problem: nn_MultiHeadAttention_75161927680550
  x_q, x_kv: [4096, 256] f32; batch_q, batch_kv: [4096] sorted int64 in [0,16)
  torch-style MHA with 8 heads, d_k=32; attention is masked to
  batch_q[i] == batch_kv[j], which (ids sorted) is block-diagonal over <=16
  segments.  Returns (out [4096,256], attn [1,8,4096,4096]).

Strategy (head-parallel SPMD over 8 cores, one head per core):
  - Host computes segment boundaries from the batch-id vectors and builds a
    Bass/Tile program specialized to them (compile happens once per boundary
    signature and is cached).
  - Device (per core h) computes, entirely in a transposed ("k-major") layout
    that needs no on-chip transposes:
      QT = Wq_h @ x_q^T + bq_h            [32, 4096]
      KT = Wk_h @ x_kv^T + bk_h           [32, 4096]
      V' = [x_kv @ Wv_h^T | 1]            [4096, 33] (ones column -> softmax
                                           denominators fall out of the AV
                                           matmul for free)
      per segment s, per 128-row kv chunk:
        scoresT = K_s_chunk @ Q_s^T       (PE, k-major)
        u = exp(scoresT / sqrt(32))       (ACT; scores are O(0.5) so the
                                           max-subtraction is unnecessary)
        DMA u -> attnT output (unnormalized, k-major)
        o'T += V'_chunk^T @ u             (PE accumulate -> [33, n_q])
      poutT = G_h @ oT[0:32]              [256, 4096] (G_h = Wo[:, 32h:32h+32])
      den   = oT[32]                      [4096]
  - Host: rinv = 1/den; scatters normalized attention blocks into a zeroed
    [8,4096,4096] (off-block entries are exactly 0 in the reference); output
    rows scale by rinv before the head-sum (diagonal scaling commutes with
    the output projection); biases bv/bo fold into a host-side constant row
    because softmax rows sum to one.
"""

import numpy as np

import concourse.bass as bass
import concourse.mybir as mybir
import concourse.tile as tile
from concourse import bacc
from concourse.bass import ts
from concourse.bass_utils import run_bass_kernel_spmd

N = 4096
D_MODEL = 256
N_HEADS = 8
D_K = 32
N_SEG = 16
N_CORES = 8
F32 = mybir.dt.float32
F32R = mybir.dt.float32r
BF16 = mybir.dt.bfloat16
# matmul datapath precision: "f32r" (~tf32, rel err ~3e-4) or "bf16"
# (faster: FWL weight loads + half the x/attn DMA bytes, rel err ~5e-3)
PRECISION = "f32r"
MDT = BF16 if PRECISION == "bf16" else F32R


# stash of the most recent BassKernelResults (test.py reads exec_time_ns)
LAST_RESULTS = None

_PROGRAM_CACHE = {}


def _segments(batch_q, batch_kv):
    """Per-segment [start, end) ranges for q and kv row spaces."""
    bq = np.asarray(batch_q).astype(np.int64)
    bk = np.asarray(batch_kv).astype(np.int64)
    assert (np.diff(bq) >= 0).all() and (np.diff(bk) >= 0).all(), (
        "batch ids must be sorted"
    )
    qb, kb = [], []
    for s in range(N_SEG):
        q0, q1 = np.searchsorted(bq, [s, s + 1])
        k0, k1 = np.searchsorted(bk, [s, s + 1])
        if q1 > q0:
            assert k1 > k0, f"segment {s} has queries but no keys (reference NaNs)"
        qb.append((int(q0), int(q1)))
        kb.append((int(k0), int(k1)))
    return tuple(qb), tuple(kb)


def _build_program(qb, kb, use_bias):
    """Build + schedule the per-core Bass program for the given boundaries."""
    kv_chunks = []  # (seg, abs_start, nck)
    seg_chunk_ids = {s: [] for s in range(N_SEG)}
    for s in range(N_SEG):
        k0, k1 = kb[s]
        off = k0
        while off < k1:
            nck = min(128, k1 - off)
            seg_chunk_ids[s].append(len(kv_chunks))
            kv_chunks.append((s, off, nck))
            off += nck
    n_kv_chunks = len(kv_chunks)
    # flat job list: (seg, qoff, qn); q chunks <=512 for psum/moving limits
    jobs = []
    attn_cols = 0
    for s in range(N_SEG):
        q0, q1 = qb[s]
        off = 0
        while off < q1 - q0:
            qn = min(512, q1 - q0 - off)
            jobs.append((s, off, qn))
            off += qn
        attn_cols = max(attn_cols, q1 - q0)
    QCOLS = attn_cols

    nc = bacc.Bacc("TRN2", target_bir_lowering=False, debug=False,
                   num_devices=N_CORES)

    xqT = nc.dram_tensor("xqT", (D_MODEL, N), MDT, kind="ExternalInput").ap()
    xkvT = nc.dram_tensor("xkvT", (D_MODEL, N), MDT, kind="ExternalInput").ap()
    # packed weights: [WqT | WkT | WvT] and [gT | bq | bk]
    wT3 = nc.dram_tensor("wT3", (D_MODEL, 9 * D_K), MDT, kind="ExternalInput").ap()
    gT = nc.dram_tensor("gT", (D_K, D_MODEL), MDT, kind="ExternalInput").ap()
    bqk = nc.dram_tensor("bqk", (4 * D_K, 2), F32, kind="ExternalInput").ap()

    attnT = nc.dram_tensor("attnT", (N, QCOLS), MDT, kind="ExternalOutput").ap()
    poutT = nc.dram_tensor("poutT", (D_MODEL, N), F32, kind="ExternalOutput").ap()
    den = nc.dram_tensor("den", (1, N), F32, kind="ExternalOutput").ap()

    SCALE = float(1.0 / np.sqrt(np.float32(D_K)))
    Exp = mybir.ActivationFunctionType.Exp
    XC = 512  # x DMA chunk width

    with tile.TileContext(nc) as tc:
        with (
            tc.tile_pool(name="big", bufs=1) as big,
            tc.tile_pool(name="wp", bufs=1) as wp,
            tc.tile_pool(name="work", bufs=12) as work,
            tc.tile_pool(name="fop", bufs=4) as fop,
            tc.tile_pool(name="psmm", bufs=6, space="PSUM") as psmm,
            tc.tile_pool(name="psacc", bufs=2, space="PSUM") as psacc,
        ):
            # ---- proj weights first; g/bqk deferred behind early x chunks
            # (g is first needed by the fin phase ~40us in; bqk only if bias)
            w3_s = wp.tile([128, 2, 9 * D_K], MDT, tag="w3")
            nc.sync.dma_start(w3_s[:], wT3.rearrange("(c p) m -> p c m", p=128))
            g_s = wp.tile([D_K, D_MODEL], MDT, tag="g")
            if use_bias:
                bqk_s = wp.tile([4 * D_K, 2], F32, tag="bqk")
                nc.scalar.dma_start(bqk_s[:], bqk[:, :])
                bq_s = bqk_s[:, 0:1]
                bk_s = bqk_s[:, 1:2]
            else:
                bq_s = bk_s = None

            # ---- x loads, chunked + interleaved so projections start early
            xq_s = big.tile([128, 2, N], MDT, tag="xq")
            xkv_s = big.tile([128, 2, N], MDT, tag="xkv")
            xqr = xqT.rearrange("(c p) n -> p c n", p=128)
            xkvr = xkvT.rearrange("(c p) n -> p c n", p=128)
            xsplits = [0, 512, 1024, 1536, 2048, 3072, 4096]
            for ci in range(len(xsplits) - 1):
                sl = slice(xsplits[ci], xsplits[ci + 1])
                nc.sync.dma_start(xq_s[:, :, sl], xqr[:, :, sl])
                nc.scalar.dma_start(xkv_s[:, :, sl], xkvr[:, :, sl])
                if ci == 1:
                    nc.scalar.dma_start(g_s[:], gT[:, :])

            # ---- Q^T, K^T projections: [32, 4096(+2 zero-pad cols)] ----
            # fp32r matmuls need an even moving-free count; odd segment
            # sizes get padded by one column, so zero the tail columns.
            qT = big.tile([128, N + 2], MDT, tag="qT")
            kT = big.tile([128, N + 2], MDT, tag="kT")
            zt = wp.tile([128, 2], F32, tag="zt")
            nc.vector.memset(zt[:], 0.0)
            nc.vector.tensor_copy(qT[:, N : N + 2], zt[:])
            nc.vector.tensor_copy(kT[:, N : N + 2], zt[:])

            vch = big.tile([128, n_kv_chunks, D_K + 1], MDT, tag="v")
            ones = wp.tile([128, 1], F32, tag="ones")
            nc.vector.memset(ones[:], 1.0)
            nc.vector.tensor_copy(
                vch[:, :, D_K : D_K + 1],
                ones[:, 0:1, None].to_broadcast((128, n_kv_chunks, 1)),
            )
            oT = big.tile([D_K + 1, N], MDT, tag="oT")
            den_sb = big.tile([1, N], F32, tag="den")
            ndma = 0

            def emit_proj(n):
                for dst, wsl, b_s, x_s in (
                    (qT, slice(0, 4 * D_K), bq_s, xq_s),
                    (kT, slice(4 * D_K, 8 * D_K), bk_s, xkv_s),
                ):
                    ps = psmm.tile([128, 512], F32, tag="mm")
                    for c in range(2):
                        nc.tensor.matmul(
                            ps[:],
                            w3_s[:, c, wsl],
                            x_s[:, c, ts(n, 512)],
                            start=(c == 0),
                            stop=(c == 1),
                        )
                    if use_bias:
                        nc.vector.tensor_scalar_add(
                            dst[:, ts(n, 512)], ps[:], b_s
                        )
                    else:
                        nc.vector.tensor_copy(dst[:, ts(n, 512)], ps[:])

            def emit_vchunk(j):
                s, a0, nck = kv_chunks[j]
                ps = psmm.tile([128, 512], F32, tag="mm")
                for c in range(2):
                    nc.tensor.matmul(
                        ps[:nck, :D_K],
                        xkv_s[:, c, a0 : a0 + nck],
                        w3_s[:, c, 8 * D_K : 9 * D_K],
                        start=(c == 0),
                        stop=(c == 1),
                    )
                nc.vector.tensor_copy(vch[:nck, j, :D_K], ps[:nck, :D_K])

            def emit_scores(job):
                s, qoff, qn = job
                q0 = qb[s][0]
                qn_p = qn + (qn & 1)
                u_tiles = []
                nonlocal ndma
                for ji, j in enumerate(seg_chunk_ids[s]):
                    _, a0, nck = kv_chunks[j]
                    rg = 32 * (ji % 4)
                    pscore = psmm.tile([128, 512], F32, tag="mm")
                    nc.tensor.matmul(
                        pscore[:nck, :qn_p],
                        kT[rg : rg + D_K, a0 : a0 + nck],
                        qT[rg : rg + D_K, q0 + qoff : q0 + qoff + qn_p],
                        start=True,
                        stop=True,
                        tile_position=(rg, 0),
                    )
                    u = work.tile([128, 512], MDT, tag="u")
                    nc.scalar.activation(
                        u[:nck, :qn_p], pscore[:nck, :qn_p], Exp, scale=SCALE
                    )
                    # keep ACT DMA-free early (exp gates AV on PE); at the
                    # tail sync carries the final pout blocks, so the last
                    # jobs' stores go to scalar instead
                    dma_eng = nc.scalar if a0 >= 3 * N // 4 else nc.sync
                    ndma += 1
                    dma_eng.dma_start(
                        attnT[a0 : a0 + nck, qoff : qoff + qn], u[:nck, :qn]
                    )
                    u_tiles.append((u, nck, j))
                return u_tiles

            def emit_av(job, u_tiles):
                s, qoff, qn = job
                q0 = qb[s][0]
                qn_p = qn + (qn & 1)
                po = psacc.tile([D_K + 1, 512], F32, tag="acc")
                last = len(u_tiles) - 1
                for ci, (u, nck, j) in enumerate(u_tiles):
                    nc.tensor.matmul(
                        po[:, :qn_p],
                        vch[:nck, j, :],
                        u[:nck, :qn_p],
                        start=(ci == 0),
                        stop=(ci == last),
                    )
                nc.vector.tensor_copy(
                    oT[:, q0 + qoff : q0 + qoff + qn], po[:, :qn]
                )
                nc.vector.tensor_copy(
                    den_sb[0:1, q0 + qoff : q0 + qoff + qn],
                    po[D_K : D_K + 1, :qn],
                )

            def emit_fin(n):
                for mo in range(2):
                    pf = psmm.tile([128, 512], F32, tag="mm")
                    nc.tensor.matmul(
                        pf[:],
                        g_s[:, mo * 128 : (mo + 1) * 128],
                        oT[:D_K, ts(n, 512)],
                        start=True,
                        stop=True,
                    )
                    ot = fop.tile([128, 512], F32, tag="fout")
                    nc.vector.tensor_copy(ot[:], pf[:])
                    peng = nc.sync if (n + mo) % 2 == 0 else nc.scalar
                    peng.dma_start(
                        poutT[mo * 128 : (mo + 1) * 128, ts(n, 512)], ot[:]
                    )

            # a fin block [512n, 512(n+1)) can run after the last AV whose
            # q-range touches it; AV for job i is emitted one step behind
            # (software pipeline), so fire fin when job i+1 has been emitted.
            last_job_touching = {}
            for i, (s, qoff, qn) in enumerate(jobs):
                q0 = qb[s][0]
                for n in range(
                    (q0 + qoff) // 512, (q0 + qoff + qn - 1) // 512 + 1
                ):
                    last_job_touching[n] = i
            DEPTH = 1  # scores run this many jobs ahead of their AV

            # interleave proj / V' / attention per 512-col block so the PE
            # always has ready work while x chunks stream in (keeps the
            # HAM clock-gate warm)
            vch_done = 0
            job_idx = 0
            pending = []
            avs_done = 0
            fin_done = set()

            def maybe_fin():
                # all blocks whose last-touching job's AV has been emitted
                for n in range(N // 512):
                    if n in fin_done:
                        continue
                    if last_job_touching.get(n, -1) < avs_done:
                        fin_done.add(n)
                        emit_fin(n)

            def push_job(job):
                nonlocal avs_done
                pending.append((job, emit_scores(job)))
                if len(pending) > DEPTH:
                    emit_av(*pending.pop(0))
                    avs_done += 1
                    maybe_fin()

            for n in range(N // 512):
                emit_proj(n)
                hi = 512 * (n + 1)
                while vch_done < n_kv_chunks:
                    s, a0, nck = kv_chunks[vch_done]
                    if a0 + nck > hi:
                        break
                    emit_vchunk(vch_done)
                    vch_done += 1
                while job_idx < len(jobs):
                    s, qoff, qn = jobs[job_idx]
                    if qb[s][1] > hi or kb[s][1] > hi:
                        break
                    push_job(jobs[job_idx])
                    job_idx += 1
            while job_idx < len(jobs):
                push_job(jobs[job_idx])
                job_idx += 1
            while pending:
                emit_av(*pending.pop(0))
                avs_done += 1
                maybe_fin()

            nc.sync.dma_start(den[0:1, :], den_sb[0:1, :])

    nc.compile()
    return nc, QCOLS


def kernel(x_q, x_kv, batch_q, batch_kv, Wq, bq, Wk, bk, Wv, bv, Wo, bo):
    global LAST_RESULTS
    x_q = np.ascontiguousarray(np.asarray(x_q, dtype=np.float32))
    x_kv = np.ascontiguousarray(np.asarray(x_kv, dtype=np.float32))
    Wq = np.asarray(Wq, dtype=np.float32)
    Wk = np.asarray(Wk, dtype=np.float32)
    Wv = np.asarray(Wv, dtype=np.float32)
    Wo = np.asarray(Wo, dtype=np.float32)
    bq = np.asarray(bq, dtype=np.float32)
    bk = np.asarray(bk, dtype=np.float32)
    bv = np.asarray(bv, dtype=np.float32)
    bo = np.asarray(bo, dtype=np.float32)

    qb, kb = _segments(batch_q, batch_kv)
    use_bias = bool(np.any(bq) or np.any(bk))
    key = (qb, kb, use_bias)
    if key not in _PROGRAM_CACHE:
        _PROGRAM_CACHE[key] = _build_program(qb, kb, use_bias)
    nc, qcols = _PROGRAM_CACHE[key]

    import ml_dtypes

    mnp = ml_dtypes.bfloat16 if PRECISION == "bf16" else np.float32
    xqT = np.ascontiguousarray(x_q.T).astype(mnp)
    xkvT = np.ascontiguousarray(x_kv.T).astype(mnp)
    in_maps = []
    for h in range(N_CORES):
        sl = slice(h * D_K, (h + 1) * D_K)
        wT3 = np.ascontiguousarray(
            np.concatenate(
                [np.tile(Wq[sl, :].T, (1, 4)), np.tile(Wk[sl, :].T, (1, 4)),
                 Wv[sl, :].T],
                axis=1,
            )
        ).astype(mnp)
        gT = np.ascontiguousarray(Wo[:, sl].T).astype(mnp)
        bqk = np.ascontiguousarray(
            np.stack([np.tile(bq[sl], 4), np.tile(bk[sl], 4)], axis=1).astype(
                np.float32
            )
        )
        in_maps.append(
            {"xqT": xqT, "xkvT": xkvT, "wT3": wT3, "gT": gT, "bqk": bqk}
        )

    res = run_bass_kernel_spmd(nc, in_maps, core_ids=list(range(N_CORES)))
    LAST_RESULTS = res

    # ---- host-side assembly ----
    out = np.zeros((N, D_MODEL), dtype=np.float32)
    attn = np.zeros((N_HEADS, N, N), dtype=np.float32)
    for h in range(N_CORES):
        r = res.results[h]
        den = r["den"].reshape(N)
        poutT = r["poutT"]  # [256, 4096]
        attnT = np.asarray(r["attnT"], dtype=np.float32)  # [4096, qcols]
        with np.errstate(divide="ignore"):
            rinv = np.where(den != 0.0, 1.0 / den, 0.0).astype(np.float32)
        out += (poutT * rinv[None, :]).T
        for s in range(N_SEG):
            q0, q1 = qb[s]
            k0, k1 = kb[s]
            if q1 <= q0 or k1 <= k0:
                continue
            blk = attnT[k0:k1, : q1 - q0].T * rinv[q0:q1, None]
            attn[h, q0:q1, k0:k1] = blk
    bo_eff = bo + bv @ Wo.T
    out += bo_eff[None, :]
    return out, attn[None]
